# revision 15
# baseline (speedup 1.0000x reference)
"""Trainium2 Bass kernel for nn_CA3RecurrentMatrix (scatter_memory).

Math: the reference's Ben-Israel-Cohen pseudoinverse iteration collapses
algebraically.  With pinv_0 = alpha*A^T, every iterate is P_n(G) A^T with
G = A^T A (C x C), and out = query @ (P_8(G) G).  On eigenvalues g of G the
map is u_8 = 1 - (1 - alpha*g)^256.  Because alpha <= 5e-4/||A||_F^2 and
g_max/||A||_F^2 ~ (sqrt(K)+sqrt(C))^2/(K*C), alpha*g_max <= ~7.2e-7, so
u_8 = 256*alpha*g to a relative accuracy of (255/2)*alpha*g ~ 9e-5 -- far
below the 2e-2 gate (and the masked early-stop never fires: the residual
stays ~||A||_F >> tol).  Hence exactly (to the required tolerance):

    out = (256*alpha) * query @ G

Distribution over 8 cores -- ZERO collectives: G is symmetric, so the column
block G[:, R_i] equals the transpose of the row block G[R_i, :] that core i
computes locally.  Sharding the OUTPUT BY COLUMNS makes the whole problem
embarrassingly parallel:

  core i:  G_rows = W_i^T A          (256 x C, W_i = A[:, R_i])   -- GEMM1
           GT = G_rows^T via PE transposes (bf16 lhsT tiles)
           out[:, R_i]^T = c1 * GT^T Q^T   (256 x B)              -- GEMM3

alpha needs ||A||_F^2 = sum(A*A); every core computes it redundantly from
the A tiles it already streams for GEMM1 (Act-engine Square with accum_out
for even k-tiles, DVE mult+reduce for odd ones), so no cross-core exchange
is needed at all.  All matmul inputs are bf16 (host-side cast; fp32 PSUM
accumulate) which keeps the tensor engine at 1 cycle/row and halves HBM
traffic.  Output is written bf16 and upcast on the host (rel-err ~5e-3
total, tolerance 2e-2).
"""
import sys, os, types

sys.path.insert(0, "/opt/trn_rl_repo")

import numpy as np

B, C, K = 8192, 2048, 4096
NCORES = 8
CB = C // NCORES     # 256-column strip of G per core
KSUP = 2             # k-subtiles per A DMA super-tile
KT = K // 128        # 32 k-tiles
NSUP = KT // KSUP    # 16 super-tiles over K
CT = C // 128        # 16 c-tiles (contraction tiles for GEMM3)
FW = 512             # free width per GEMM3 f-block
FB = B // FW         # 16 f-blocks over the batch
MT = CB // 128       # 2 output row tiles per core
NB = C // 512        # 4 n-blocks in GEMM1

_CACHE = {}


def _install_ntff_shim():
    """Make trace=True work under axon (antenv.axon_hooks is absent here)."""
    if "antenv.axon_hooks" in sys.modules:
        return
    try:
        import antenv
    except ImportError:
        return
    mod = types.ModuleType("antenv.axon_hooks")
    state = {"hook": None, "resolved": False}

    def set_axon_ntff_profile_hook(hook):
        state["hook"], state["resolved"] = hook, True

    def get_axon_ntff_profile_hook():
        if not state["resolved"]:
            state["resolved"] = True
            try:
                if "/root/.axon_site" not in sys.path:
                    sys.path.insert(0, "/root/.axon_site")
                from trn_agent_boot.trn_boot import _ntff_profile_via_ctypes
                state["hook"] = _ntff_profile_via_ctypes("/opt/axon/libaxon_pjrt.so")
            except Exception:
                state["hook"] = None
        return state["hook"]

    mod.set_axon_ntff_profile_hook = set_axon_ntff_profile_hook
    mod.get_axon_ntff_profile_hook = get_axon_ntff_profile_hook
    sys.modules["antenv.axon_hooks"] = mod
    antenv.axon_hooks = mod


def build_nc():
    import concourse.bacc as bacc
    import concourse.mybir as mybir
    from concourse import tile, bass_isa

    f32 = mybir.dt.float32
    bf16 = mybir.dt.bfloat16

    nc = bacc.Bacc("TRN2", target_bir_lowering=False, debug=False,
                   num_devices=NCORES)
    # a: A rows regrouped so one dma_start fills a [128, KSUP*C] super-tile:
    #    row s*128+p, col t*C+c  <-  A[s*(128*KSUP) + t*128 + p, c]
    a_d = nc.dram_tensor("a", (NSUP * 128, KSUP * C), bf16, kind="ExternalInput")
    # w: whole strip W_i = A[:, R_i] keyed [p, k_tile*CB + r]
    w_d = nc.dram_tensor("w", (128, KT * CB), bf16, kind="ExternalInput")
    # qt: Q^T regrouped per f-block: row f*128+p, col t*FW+j <- Q[f*FW+j, t*128+p]
    qt_d = nc.dram_tensor("qt", (FB * 128, CT * FW), bf16, kind="ExternalInput")
    ls_d = nc.dram_tensor("ls", (128, 1), f32, kind="ExternalInput")
    id_d = nc.dram_tensor("ident", (128, 128), f32, kind="ExternalInput")
    # out: row m*128+p = r, col b  ->  out_full[b, i*CB + r]
    out_d = nc.dram_tensor("out", (CB, B), bf16, kind="ExternalOutput")

    ALPHA_CLAMP = 5e-4
    C1 = 256.0

    with tile.TileContext(nc) as tc:
        with tc.tile_pool(name="sbuf", bufs=1) as pool, \
             tc.tile_pool(name="psum", bufs=1, space="PSUM") as psum:
            acc = pool.tile([128, KT], f32, tag="acc")

            # whole W strip resident; first eighth loaded first so the
            # first matmul is gated only by ~0.25 MB of DMA
            wk = pool.tile([128, KT * CB], bf16, tag="wk")
            nc.gpsimd.dma_start(wk[:, :4 * CB], w_d.ap()[:, :4 * CB])

            # G-row accumulators: bank (m*NB+n) holds G[R_i half m, n-block]
            pg = [psum.tile([128, 512], f32, tag=f"ps{j}", name=f"pg{j}")
                  for j in range(8)]

            # ---- GEMM1: G_rows = W_i^T A  [CB, C]; fro2 accum on Act+DVE ----
            with nc.named_scope("gemm1"):
                for s in range(NSUP):
                    ak = pool.tile([128, KSUP * C], bf16, tag="ak", bufs=6)
                    if s == 0:
                        # fine-grained first super: 8 x 0.25MB across both
                        # queues so the first matmul is gated minimally
                        for t in range(KSUP):
                            for n in range(NB):
                                lo = t * C + n * 512
                                (nc.sync if n % 2 == 0 else nc.scalar).dma_start(
                                    ak[:, lo:lo + 512],
                                    a_d.ap()[:128, lo:lo + 512])
                    else:
                        (nc.sync if s % 2 == 0 else nc.scalar).dma_start(
                            ak[:], a_d.ap()[s * 128:(s + 1) * 128, :])
                    if s == 1:
                        # remaining W, then deferred small inputs (Pool queue)
                        for j in range(1, 8):
                            nc.gpsimd.dma_start(
                                wk[:, j * 4 * CB:(j + 1) * 4 * CB],
                                w_d.ap()[:, j * 4 * CB:(j + 1) * 4 * CB])
                        ls_sb = pool.tile([128, 1], f32, tag="ls")
                        nc.gpsimd.dma_start(ls_sb[:], ls_d.ap()[:, :])
                        ident_sb = pool.tile([128, 128], f32, tag="ident")
                        nc.gpsimd.dma_start(ident_sb[:], id_d.ap()[:, :])
                    for t in range(KSUP):
                        k = s * KSUP + t
                        mn_order = ([(m, n) for n in range(NB) for m in range(MT)]
                                    if s == 0 else
                                    [(m, n) for m in range(MT) for n in range(NB)])
                        for m, n in mn_order:
                            nc.tensor.matmul(
                                pg[m * NB + n][:],
                                wk[:, k * CB + m * 128: k * CB + (m + 1) * 128],
                                ak[:, t * C + n * 512: t * C + (n + 1) * 512],
                                start=(k == 0), stop=(k == KT - 1))
                        if k % 2 == 0:
                            sq = pool.tile([128, C], bf16, tag="sq", bufs=2)
                            nc.scalar.activation(
                                sq[:], ak[:, t * C:(t + 1) * C],
                                mybir.ActivationFunctionType.Square,
                                accum_out=acc[:, k:k + 1])
                        else:
                            sqv = pool.tile([128, C], bf16, tag="sqv", bufs=2)
                            nc.vector.tensor_mul(
                                sqv[:], ak[:, t * C:(t + 1) * C],
                                ak[:, t * C:(t + 1) * C])
                            nc.vector.reduce_sum(acc[:, k:k + 1], sqv[:],
                                                 axis=mybir.AxisListType.X)

            # ---- evacuate G_rows per n-block (DVE: m=0, Act: m=1) so the
            # first transpose unblocks after the FIRST psum copy ----
            g_rows = []
            for m in range(MT):
                grn = []
                for n in range(NB):
                    gr = pool.tile([128, 512], f32, tag=f"grows{m}_{n}",
                                   name=f"grows{m}_{n}")
                    if m == 0:
                        nc.vector.tensor_copy(gr[:], pg[m * NB + n][:])
                    else:
                        nc.scalar.activation(
                            gr[:], pg[m * NB + n][:],
                            mybir.ActivationFunctionType.Copy)
                    grn.append(gr)
                g_rows.append(grn)

            # fro2 pieces (cheap, needed later for the alpha chain)
            asum = pool.tile([128, 1], f32, tag="asum")
            nc.vector.reduce_sum(asum[:], acc[:], axis=mybir.AxisListType.X)
            fro2 = pool.tile([128, 1], f32, tag="fro2")
            nc.gpsimd.partition_all_reduce(
                fro2[:], asum[:], channels=128,
                reduce_op=bass_isa.ReduceOp.add)
            ex = pool.tile([128, 1], f32, tag="ex")
            nc.scalar.activation(ex[:], ls_sb[:],
                                 mybir.ActivationFunctionType.Exp)

            def alpha_tail():
                # rest of alpha chain: c1 = 256*min(e^ls,clamp)/(fro2+1e-8)
                emin = pool.tile([128, 1], f32, tag="emin")
                nc.vector.tensor_scalar_min(emin[:], ex[:], ALPHA_CLAMP)
                den = pool.tile([128, 1], f32, tag="den")
                nc.vector.tensor_scalar_add(den[:], fro2[:], 1e-8)
                r0 = pool.tile([128, 1], f32, tag="r0")
                nc.vector.reciprocal(r0[:], den[:])
                # one Newton step: r = r0*(2 - den*r0)
                t1 = pool.tile([128, 1], f32, tag="t1")
                nc.vector.tensor_mul(t1[:], den[:], r0[:])
                t2 = pool.tile([128, 1], f32, tag="t2")
                nc.vector.tensor_scalar(t2[:], t1[:], -1.0, 2.0,
                                        op0=mybir.AluOpType.mult,
                                        op1=mybir.AluOpType.add)
                rr = pool.tile([128, 1], f32, tag="rr")
                nc.vector.tensor_mul(rr[:], r0[:], t2[:])
                al = pool.tile([128, 1], f32, tag="al")
                nc.vector.tensor_mul(al[:], emin[:], rr[:])
                c1b = pool.tile([128, 1], f32, tag="c1b")
                nc.vector.tensor_scalar_mul(c1b[:], al[:], C1)
                return c1b

            # ---- GEMM3: out^T[R_i, :] = c1 * GT^T Q^T  [CB, B] ----
            # PE transposes of G_rows (-> GT lhsT tiles, banks ps4..ps7) are
            # interleaved into the first f-block so the PE never idles.
            gt = [pool.tile([128, CB], bf16, tag=f"gt{t}", name=f"gtt{t}")
                  for t in range(CT)]
            c1b = None
            with nc.named_scope("gemm3"):
                ost = [pool.tile([128, B], bf16, tag=f"ost{m}", name=f"ost{m}")
                       for m in range(MT)]
                for f in range(FB):
                    qtf = pool.tile([128, CT * FW], bf16, tag="qtf", bufs=3)
                    (nc.sync if f % 2 == 0 else nc.scalar).dma_start(
                        qtf[:], qt_d.ap()[f * 128:(f + 1) * 128, :])
                    pos = [psum.tile([128, FW], f32, tag=f"ps{2 * (f % 2) + m}",
                                     name=f"po{f}_{m}") for m in range(MT)]
                    for t in range(CT):
                        if f == 0:
                            for m in range(MT):
                                tp = psum.tile(
                                    [128, 128], f32,
                                    tag=f"ps{4 + (t * MT + m) % 4}",
                                    name=f"tp{t}_{m}")
                                nc.tensor.transpose(
                                    tp[:],
                                    g_rows[m][t // 4][:, (t % 4) * 128:
                                                      (t % 4) * 128 + 128],
                                    ident_sb[:])
                                nc.vector.tensor_copy(
                                    gt[t][:, m * 128:(m + 1) * 128], tp[:])
                            if t == 1:
                                c1b = alpha_tail()
                        for m in range(MT):
                            nc.tensor.matmul(
                                pos[m][:],
                                gt[t][:, m * 128:(m + 1) * 128],
                                qtf[:, t * FW:(t + 1) * FW],
                                start=(t == 0), stop=(t == CT - 1))
                    for m in range(MT):
                        nc.vector.tensor_scalar_mul(
                            ost[m][:, f * FW:(f + 1) * FW], pos[m][:], c1b[:])
                    if f % 2 == 1:
                        lo, hi = (f - 1) * FW, (f + 1) * FW
                        for m in range(MT):
                            nc.gpsimd.dma_start(
                                out_d.ap()[m * 128:(m + 1) * 128, lo:hi],
                                ost[m][:, lo:hi])
    nc.compile()
    return nc


def _get_nc():
    if "nc" not in _CACHE:
        _CACHE["nc"] = build_nc()
    return _CACHE["nc"]


def _host_prep(query, memory_mean, ben_israel_log_scale):
    import ml_dtypes
    bf = ml_dtypes.bfloat16

    a32 = np.ascontiguousarray(np.asarray(memory_mean, dtype=np.float32))
    q32 = np.asarray(query, dtype=np.float32)
    abf = a32.astype(bf)
    # A regrouped: [s, t, p, c] -> [s, p, t, c] -> (NSUP*128, KSUP*C)
    a_arr = np.ascontiguousarray(
        abf.reshape(NSUP, KSUP, 128, C).transpose(0, 2, 1, 3)
    ).reshape(NSUP * 128, KSUP * C)
    qbf = q32.astype(bf)
    # Q^T regrouped per f-block: Q[f*FW+j, t*128+p] -> row f*128+p, col t*FW+j
    qt_arr = np.ascontiguousarray(
        qbf.reshape(FB, FW, CT, 128).transpose(0, 3, 2, 1)
    ).reshape(FB * 128, CT * FW)
    ls = np.full((128, 1), np.asarray(ben_israel_log_scale, dtype=np.float32))
    ident = np.eye(128, dtype=np.float32)

    in_maps = []
    for i in range(NCORES):
        wbf = np.ascontiguousarray(abf[:, i * CB:(i + 1) * CB])
        # strip keyed [p, k_tile*CB + r]
        w_arr = np.ascontiguousarray(
            wbf.reshape(KT, 128, CB).transpose(1, 0, 2)
        ).reshape(128, KT * CB)
        in_maps.append({"a": a_arr, "w": w_arr, "qt": qt_arr, "ls": ls,
                        "ident": ident})
    return in_maps


def _run(query, memory_mean, ben_israel_log_scale, trace=False, trace_cores=None):
    from concourse import bass_utils

    _install_ntff_shim()
    nc = _get_nc()
    in_maps = _host_prep(query, memory_mean, ben_israel_log_scale)
    res = bass_utils.run_bass_kernel_spmd(
        nc, in_maps, core_ids=list(range(NCORES)), trace=trace,
        trace_cores=trace_cores)
    out = np.empty((B, C), dtype=np.float32)
    for i in range(NCORES):
        out[:, i * CB:(i + 1) * CB] = \
            np.asarray(res.results[i]["out"]).astype(np.float32).T
    return out, res


def kernel(query, memory_mean, ben_israel_log_scale):
    out, _ = _run(query, memory_mean, ben_israel_log_scale, trace=False)
    return out
